# revision 6
# baseline (speedup 1.0000x reference)
"""Trainium2 Bass kernel for nn_DeepGraphInfomax (vq_codebook).

Computes, for x [N,D], W_enc [D,D], comm_labels [N] (N=100000, D=512, K=128):
  pos_z = l2norm_rows(x @ W_enc)            [N, D]
  mu    = segment_mean(pos_z, comm_labels)  [K, D]
  dist  = pos_z @ mu.T                      [N, K]
  r     = softmax(30 * dist, axis=1)        [N, K]
  u     = segment_mean(pos_z, argmax dist)  [K, D]

Sharding: rows (N) split across 8 NeuronCores (padded to 12544 rows/core);
segment sums become per-core partial sums + AllReduce over [K, D+1];
W_enc and mu replicated.  x is fed pre-transposed ([D, rows]) so the
contraction dim lands on partitions without on-chip transposes.

Self-contained: hardcodes all shapes; only imports the concourse/bass stack
available in the environment.
"""
from contextlib import ExitStack

import numpy as np

import concourse.bass as bass
import concourse.mybir as mybir
import concourse.tile as tile
from concourse import bacc
from concourse.bass_utils import run_bass_kernel_spmd
from concourse.masks import make_identity

P = 128          # partitions
D = 512          # feature dim
K = 128          # clusters
TEMP = 30.0
NCORES = 8
N = 100000
SH = 12544       # rows per core (padded): 8*12544 = 100352
C = SH // P      # 98 chunks of 128 rows
NPAD = NCORES * SH
J = D // P       # 4 contraction blocks
GS = 4           # chunks per group (DMA batching / distT width)
RES_GROUPS = 16  # trailing groups of pos_z kept resident in SBUF for pass 2

f32 = mybir.dt.float32
f32r = mybir.dt.float32r
i32 = mybir.dt.int32
ALU = mybir.AluOpType
ACTF = mybir.ActivationFunctionType
AX = mybir.AxisListType

# groups: (start_chunk, n_chunks)
GROUPS = []
_c = 0
while _c < C:
    GROUPS.append((_c, min(GS, C - _c)))
    _c += GS
NG = len(GROUPS)


def build(repeat=None, collectives=True):
    """Build + compile the per-core program.

    repeat: if set, wrap the whole body in a hardware loop executing it
    `repeat` times (timing variant; implies collectives replaced by copies).
    """
    nc = bacc.Bacc("TRN2", target_bir_lowering=False, debug=False,
                   num_devices=NCORES)

    xt_d = nc.dram_tensor("xt", [D, SH], f32, kind="ExternalInput")
    w_d = nc.dram_tensor("w", [D, D], f32, kind="ExternalInput")
    lab_d = nc.dram_tensor("labels", [P, C], i32, kind="ExternalInput")

    poz_d = nc.dram_tensor("pos_z", [SH, D], f32, kind="ExternalOutput")
    dist_d = nc.dram_tensor("dist", [SH, K], f32, kind="ExternalOutput")
    r_d = nc.dram_tensor("r", [SH, K], f32, kind="ExternalOutput")
    mu_d = nc.dram_tensor("mu", [K, D], f32, kind="ExternalOutput")
    u_d = nc.dram_tensor("u", [K, D], f32, kind="ExternalOutput")

    with tile.TileContext(nc) as tc, ExitStack() as ctx:
        const = ctx.enter_context(tc.tile_pool(name="const", bufs=1))

        # --- constants / setup ---
        iota_f = const.tile([P, K], f32, name="iota_f")
        nc.gpsimd.iota(iota_f[:], pattern=[[1, K]], base=0, channel_multiplier=0,
                       allow_small_or_imprecise_dtypes=True)
        id_f = const.tile([P, P], f32, name="id_f")
        make_identity(nc, id_f[:])
        id_r = const.tile([P, P], f32r, name="id_r")
        nc.vector.tensor_copy(id_r[:], id_f[:])
        ones_sb = const.tile([P, 1], f32, name="ones_sb")
        nc.vector.memset(ones_sb[:], 1.0)

        lab_i = const.tile([P, C], i32, name="lab_i")
        nc.sync.dma_start(lab_i[:], lab_d.ap())
        lab_f = const.tile([P, C], f32, name="lab_f")
        nc.vector.tensor_copy(lab_f[:], lab_i[:])
        valid = const.tile([P, C], f32, name="valid")
        nc.vector.tensor_scalar(out=valid[:], in0=lab_f[:], scalar1=float(K),
                                scalar2=None, op0=ALU.is_lt)
        sq_scr = const.tile([P, D], f32, name="sq_scr")

        def body():
            with ExitStack() as bctx:
                _body(bctx, tc)

        def _body(bctx, tc):
            # resident pos_z group tiles: trailing RES_GROUPS stay valid
            resv = bctx.enter_context(tc.tile_pool(name="resv", bufs=RES_GROUPS))
            acc = bctx.enter_context(tc.tile_pool(name="acc", bufs=1, space="PSUM"))
            mid = bctx.enter_context(tc.tile_pool(name="mid", bufs=1))
            dram = bctx.enter_context(tc.tile_pool(name="dram", bufs=1, space="DRAM"))

            # ---------------- pass 1 ----------------
            mu_ps = acc.tile([P, D], f32, name="mu_ps", tag="accbig")
            cnt_ps = acc.tile([P, 1], f32, name="cnt_ps", tag="acccnt")

            poz_tiles = {}
            prev = None  # pending (oh, poz_slice) segment-accumulate

            def seg_acc(c_first, c_last):
                oh_p, poz_p = prev
                nc.tensor.matmul(mu_ps[:], oh_p[:], poz_p, start=c_first, stop=c_last)
                nc.tensor.matmul(cnt_ps[:], oh_p[:].bitcast(f32), ones_sb[:],
                                 start=c_first, stop=c_last)

            with ExitStack() as p1ctx:
                p1 = p1ctx.enter_context(tc.tile_pool(name="p1", bufs=3))
                xtp = p1ctx.enter_context(tc.tile_pool(name="xtp", bufs=2))
                yps = p1ctx.enter_context(tc.tile_pool(name="yps", bufs=2, space="PSUM"))

                w_sb = p1.tile([P, J, D], f32r, name="w_sb", bufs=1)
                nc.sync.dma_start(
                    w_sb[:], w_d.ap().rearrange("(j p) n -> p j n", p=P).bitcast(f32r))

                for g, (c0, gs) in enumerate(GROUPS):
                    xt_g = xtp.tile([P, J, GS * P], f32r, name="xt_g")
                    nc.sync.dma_start(
                        xt_g[:, :, :gs * P],
                        xt_d.ap()[:, c0 * P:(c0 + gs) * P]
                        .rearrange("(j p) n -> p j n", p=P).bitcast(f32r))
                    poz_g = resv.tile([P, GS, D], f32r, name="poz_g")
                    poz_tiles[g] = poz_g
                    for cl in range(gs):
                        c = c0 + cl
                        y_ps = yps.tile([P, D], f32, name="y_ps")
                        for j in range(J):
                            nc.tensor.matmul(y_ps[:],
                                             xt_g[:, j, cl * P:(cl + 1) * P],
                                             w_sb[:, j, :],
                                             start=(j == 0), stop=(j == J - 1))
                        if prev is not None:
                            seg_acc(c == 1, False)
                        # row l2 normalization
                        norm2 = p1.tile([P, 1], f32, name="norm2")
                        nc.scalar.activation(sq_scr[:], y_ps[:], ACTF.Square,
                                             accum_out=norm2[:])
                        nc.vector.tensor_scalar_max(norm2[:], norm2[:], 1e-30)
                        nrm = p1.tile([P, 1], f32, name="nrm")
                        nc.scalar.sqrt(nrm[:], norm2[:])
                        rnorm = p1.tile([P, 1], f32, name="rnorm")
                        nc.vector.reciprocal(rnorm[:], nrm[:])
                        nc.vector.tensor_scalar(out=poz_g[:, cl, :], in0=y_ps[:],
                                                scalar1=rnorm[:], scalar2=None,
                                                op0=ALU.mult)
                        oh = p1.tile([P, K], f32r, name="oh")
                        nc.vector.tensor_scalar(out=oh[:], in0=iota_f[:],
                                                scalar1=lab_f[:, c:c + 1],
                                                scalar2=None, op0=ALU.is_equal)
                        prev = (oh, poz_g[:, cl, :])
                    nc.sync.dma_start(
                        poz_d.ap()[c0 * P:(c0 + gs) * P, :]
                        .rearrange("(c p) d -> p c d", p=P),
                        poz_g[:, :gs, :].bitcast(f32))
                seg_acc(False, True)
                prev = None

            # ---------------- mu finalize + AllReduce ----------------
            mu_cat = mid.tile([P, D + 1], f32, name="mu_cat")
            nc.scalar.copy(mu_cat[:, :D], mu_ps[:])
            nc.vector.tensor_copy(mu_cat[:, D:], cnt_ps[:])
            ar1_in = dram.tile([P, D + 1], f32, name="ar1_in")
            ar1_out = dram.tile([P, D + 1], f32, name="ar1_out")
            nc.gpsimd.dma_start(ar1_in[:], mu_cat[:])
            if collectives:
                nc.gpsimd.collective_compute(
                    "AllReduce", ALU.add,
                    replica_groups=[list(range(NCORES))],
                    ins=[ar1_in.opt()], outs=[ar1_out.opt()])
            else:
                nc.gpsimd.dma_start(ar1_out[:], ar1_in[:])
            arred = mid.tile([P, D + 1], f32, name="arred")
            nc.sync.dma_start(arred[:], ar1_out[:])

            cntc = mid.tile([P, 1], f32, name="cntc")
            nc.vector.tensor_scalar_max(cntc[:], arred[:, D:], 1.0)
            rcnt = mid.tile([P, 1], f32, name="rcnt")
            nc.vector.reciprocal(rcnt[:], cntc[:])
            mu_f = mid.tile([P, D], f32, name="mu_f")
            nc.vector.tensor_scalar(out=mu_f[:], in0=arred[:, :D], scalar1=rcnt[:],
                                    scalar2=None, op0=ALU.mult)
            nc.sync.dma_start(mu_d.ap(), mu_f[:])
            mu_r = mid.tile([P, D], f32r, name="mu_r")
            nc.vector.tensor_copy(mu_r[:], mu_f[:])

            # ---------------- pass 2 ----------------
            with ExitStack() as p2ctx:
                tps = p2ctx.enter_context(tc.tile_pool(name="tps", bufs=4, space="PSUM"))
                p2 = p2ctx.enter_context(tc.tile_pool(name="p2", bufs=3))
                pzt = p2ctx.enter_context(tc.tile_pool(name="pzt", bufs=2))
                stg = p2ctx.enter_context(tc.tile_pool(name="stg", bufs=2))
                dps = p2ctx.enter_context(tc.tile_pool(name="dps", bufs=2, space="PSUM"))

                # transpose mu -> muT blocks [d, k]
                muT = mid.tile([P, J, P], f32r, name="muT")
                for j in range(J):
                    mt_ps = tps.tile([P, P], f32r, name="mt_ps", tag="tp")
                    nc.tensor.transpose(mt_ps[:], mu_r[:, j * P:(j + 1) * P], id_r[:])
                    nc.vector.tensor_copy(muT[:, j, :], mt_ps[:])

                u_ps = acc.tile([P, D], f32, name="u_ps", tag="accbig")
                cnt2_ps = acc.tile([P, 1], f32, name="cnt2_ps", tag="acccnt")

                state = {}   # per-group staged tiles
                useg = [0]

                def stage_a(g):
                    """load/locate poz, transpose to pozT, distT matmuls"""
                    c0, gs = GROUPS[g]
                    if g >= NG - RES_GROUPS:
                        poz_g = poz_tiles[g]
                    else:
                        poz_g = p2.tile([P, GS, D], f32r, name="poz_rr", bufs=2)
                        nc.sync.dma_start(
                            poz_g[:, :gs, :],
                            poz_d.ap()[c0 * P:(c0 + gs) * P, :]
                            .rearrange("(c p) d -> p c d", p=P).bitcast(f32r))
                    pozT = pzt.tile([P, J, GS * P], f32r, name="pozT")
                    for cl in range(gs):
                        for j in range(J):
                            pt_ps = tps.tile([P, P], f32r, name="pt_ps", tag="tp")
                            nc.tensor.transpose(pt_ps[:],
                                                poz_g[:, cl, j * P:(j + 1) * P],
                                                id_r[:])
                            eng = nc.vector if (j % 2 == 0) else nc.scalar
                            if eng is nc.vector:
                                nc.vector.tensor_copy(pozT[:, j, cl * P:(cl + 1) * P],
                                                      pt_ps[:])
                            else:
                                nc.scalar.copy(pozT[:, j, cl * P:(cl + 1) * P],
                                               pt_ps[:])
                    dT_ps = dps.tile([P, GS * P], f32, name="dT_ps", tag="dT")
                    for j in range(J):
                        nc.tensor.matmul(dT_ps[:, :gs * P], muT[:, j, :],
                                         pozT[:, j, :gs * P],
                                         start=(j == 0), stop=(j == J - 1))
                    dT_sb = p2.tile([P, GS * P], f32, name="dT_sb")
                    nc.vector.tensor_copy(dT_sb[:, :gs * P], dT_ps[:, :gs * P])
                    state[g] = (poz_g, dT_sb)

                def stage_b(g):
                    """per-chunk: re-transpose dist, softmax, oh2; group DMAs"""
                    c0, gs = GROUPS[g]
                    poz_g, dT_sb = state.pop(g)
                    dist_g = stg.tile([P, GS, K], f32, name="dist_g")
                    r_g = stg.tile([P, GS, K], f32, name="r_g")
                    oh2s = []
                    for cl in range(gs):
                        c = c0 + cl
                        d_ps = tps.tile([P, P], f32, name="d_ps", tag="tp")
                        nc.tensor.transpose(d_ps[:],
                                            dT_sb[:, cl * P:(cl + 1) * P], id_f[:])
                        dist_sb = dist_g[:, cl, :]
                        nc.scalar.copy(dist_sb, d_ps[:])
                        rmax = p2.tile([P, 1], f32, name="rmax")
                        nc.vector.reduce_max(rmax[:], dist_sb, axis=AX.X)
                        negb = p2.tile([P, 1], f32, name="negb")
                        nc.vector.tensor_scalar_mul(negb[:], rmax[:], -TEMP)
                        sumexp = p2.tile([P, 1], f32, name="sumexp")
                        nc.scalar.activation(r_g[:, cl, :], dist_sb, ACTF.Exp,
                                             bias=negb[:], scale=TEMP,
                                             accum_out=sumexp[:])
                        rsum = p2.tile([P, 1], f32, name="rsum")
                        nc.vector.reciprocal(rsum[:], sumexp[:])
                        nc.vector.tensor_scalar(out=r_g[:, cl, :], in0=r_g[:, cl, :],
                                                scalar1=rsum[:], scalar2=None,
                                                op0=ALU.mult)
                        oh2 = p2.tile([P, K], f32r, name="oh2")
                        nc.vector.tensor_scalar(out=oh2[:], in0=dist_sb,
                                                scalar1=rmax[:],
                                                scalar2=valid[:, c:c + 1],
                                                op0=ALU.is_equal, op1=ALU.mult)
                        oh2s.append(oh2)
                    nc.sync.dma_start(
                        dist_d.ap()[c0 * P:(c0 + gs) * P, :]
                        .rearrange("(c p) k -> p c k", p=P), dist_g[:, :gs, :])
                    nc.sync.dma_start(
                        r_d.ap()[c0 * P:(c0 + gs) * P, :]
                        .rearrange("(c p) k -> p c k", p=P), r_g[:, :gs, :])
                    state[(g, "u")] = (poz_g, oh2s)

                def stage_u(g, last):
                    c0, gs = GROUPS[g]
                    poz_g, oh2s = state.pop((g, "u"))
                    for cl in range(gs):
                        first = useg[0] == 0
                        useg[0] += 1
                        is_last = last and cl == gs - 1
                        nc.tensor.matmul(u_ps[:], oh2s[cl][:], poz_g[:, cl, :],
                                         start=first, stop=is_last)
                        nc.tensor.matmul(cnt2_ps[:], oh2s[cl][:].bitcast(f32),
                                         ones_sb[:], start=first, stop=is_last)

                for g in range(NG):
                    stage_a(g)
                    if g >= 1:
                        stage_b(g - 1)
                    if g >= 2:
                        stage_u(g - 2, False)
                stage_b(NG - 1)
                stage_u(NG - 2, False)
                stage_u(NG - 1, True)

                # ---------------- u finalize + AllReduce ----------------
                u_cat = mid.tile([P, D + 1], f32, name="u_cat")
                nc.scalar.copy(u_cat[:, :D], u_ps[:])
                nc.vector.tensor_copy(u_cat[:, D:], cnt2_ps[:])
                ar2_in = dram.tile([P, D + 1], f32, name="ar2_in")
                ar2_out = dram.tile([P, D + 1], f32, name="ar2_out")
                nc.gpsimd.dma_start(ar2_in[:], u_cat[:])
                if collectives:
                    nc.gpsimd.collective_compute(
                        "AllReduce", ALU.add,
                        replica_groups=[list(range(NCORES))],
                        ins=[ar2_in.opt()], outs=[ar2_out.opt()])
                else:
                    nc.gpsimd.dma_start(ar2_out[:], ar2_in[:])
                arred2 = mid.tile([P, D + 1], f32, name="arred2")
                nc.sync.dma_start(arred2[:], ar2_out[:])
                cnt2c = mid.tile([P, 1], f32, name="cnt2c")
                nc.vector.tensor_scalar_max(cnt2c[:], arred2[:, D:], 1.0)
                rcnt2 = mid.tile([P, 1], f32, name="rcnt2")
                nc.vector.reciprocal(rcnt2[:], cnt2c[:])
                u_f = mid.tile([P, D], f32, name="u_f")
                nc.vector.tensor_scalar(out=u_f[:], in0=arred2[:, :D],
                                        scalar1=rcnt2[:], scalar2=None,
                                        op0=ALU.mult)
                nc.sync.dma_start(u_d.ap(), u_f[:])

        if repeat is not None:
            with tc.For_i(0, repeat, 1):
                body()
        else:
            body()

    nc.compile()
    return nc


_CACHE = {}


def _compiled():
    if "nc" not in _CACHE:
        _CACHE["nc"] = build()
    return _CACHE["nc"]


def _shard_inputs(x, W_enc, comm_labels):
    x = np.asarray(x, dtype=np.float32)
    W_enc = np.asarray(W_enc, dtype=np.float32)
    lab = np.asarray(comm_labels, dtype=np.int32)
    xp = np.zeros((NPAD, D), np.float32)
    xp[:N] = x
    labp = np.full((NPAD,), K, np.int32)
    labp[:N] = lab
    in_maps = []
    for i in range(NCORES):
        sl = slice(i * SH, (i + 1) * SH)
        in_maps.append({
            "xt": np.ascontiguousarray(xp[sl].T),
            "w": W_enc,
            "labels": np.ascontiguousarray(labp[sl].reshape(C, P).T),
        })
    return in_maps


def kernel(x, W_enc, comm_labels):
    nc = _compiled()
    in_maps = _shard_inputs(x, W_enc, comm_labels)
    res = run_bass_kernel_spmd(nc, in_maps, core_ids=list(range(NCORES)))
    outs = res.results
    pos_z = np.concatenate([outs[i]["pos_z"] for i in range(NCORES)], axis=0)[:N]
    dist = np.concatenate([outs[i]["dist"] for i in range(NCORES)], axis=0)[:N]
    r = np.concatenate([outs[i]["r"] for i in range(NCORES)], axis=0)[:N]
    mu = outs[0]["mu"]
    u = outs[0]["u"]
    return pos_z, mu, r, dist, u
